# revision 3
# baseline (speedup 1.0000x reference)
"""Trainium2 Bass kernel for nn_Decoder (MusicVAE-style hierarchical decoder).

Strategy (8 NeuronCores, data-parallel over batch, no inter-core comms):
  - Conductor LSTM (16 sequential levels, batch 32/core) computes per-level
    embeddings.
  - Decoder levels are INDEPENDENT (initial state from dec_h0/dec_c0,
    note0=0), so all 16 levels are batched: effective decoder batch
    16*32 = 512 rows per core, 16 sequential note steps.
  - The conductor embedding is constant within a level, so its gate
    contribution (emb @ Wih[:, :H].T + bias) is precomputed once ("ge").
  - Everything lives feature-major: [features on partitions, rows free].
    Weights are the stationary matmul operand, activations stream.
  - bf16 matmuls with fp32 PSUM accumulation; c state in fp32.
"""
import numpy as np
import ml_dtypes

import concourse.bacc as bacc
import concourse.tile as tile
import concourse.mybir as mybir
from concourse.bass_utils import run_bass_kernel_spmd

bf16 = ml_dtypes.bfloat16
F32 = mybir.dt.float32
BF = mybir.dt.bfloat16
AF = mybir.ActivationFunctionType

NCORES = 8
B, Z, H, T = 256, 512, 1024, 512
L, NS = 16, 16
Bc = B // NCORES            # 32 batch rows per core
R = L * Bc                  # 512 decoder rows per core (levels x batch)
HK, TK, ZK = H // 128, T // 128, Z // 128   # 8, 4, 4
G = 4 * H // 128            # 32 gate chunks of 128


def _declare(nc):
    d = {}
    ei = dict(kind="ExternalInput")
    d["ident"] = nc.dram_tensor("ident", [128, 128], BF, **ei)
    d["ones"] = nc.dram_tensor("ones", [1, R], BF, **ei)
    d["cbias"] = nc.dram_tensor("cbias", [1, 4 * H], BF, **ei)
    d["dbias"] = nc.dram_tensor("dbias", [1, 4 * H], BF, **ei)
    d["obias"] = nc.dram_tensor("obias", [128, TK], F32, **ei)
    d["zT"] = nc.dram_tensor("zT", [128, ZK, R], BF, **ei)
    d["h0T"] = nc.dram_tensor("h0T", [128, HK, R], BF, **ei)
    d["c0T"] = nc.dram_tensor("c0T", [128, HK, R], F32, **ei)
    d["cwih"] = nc.dram_tensor("cwih", [128, ZK, 4 * H], BF, **ei)
    d["cwhh"] = nc.dram_tensor("cwhh", [128, HK, 4 * H], BF, **ei)
    d["dwe"] = nc.dram_tensor("dwe", [HK, G, 128, 128], BF, **ei)
    d["dwn"] = nc.dram_tensor("dwn", [128, TK, 4 * H], BF, **ei)
    d["dwhh"] = nc.dram_tensor("dwhh", [128, HK, 4 * H], BF, **ei)
    d["owt"] = nc.dram_tensor("owt", [128, HK, T], BF, **ei)
    d["outbuf"] = nc.dram_tensor("outbuf", [NS, TK, 128, R], BF,
                                 kind="ExternalOutput")
    return d


def _body(nc, tc, d):
    import contextlib
    with contextlib.ExitStack() as ctx:
        Pp = ctx.enter_context(tc.tile_pool(name="persist", bufs=1))

        t_ident = Pp.tile([128, 128], BF, tag="ident")
        nc.sync.dma_start(t_ident[:], d["ident"][:])
        t_ones = Pp.tile([1, R], BF, tag="ones")
        nc.sync.dma_start(t_ones[:], d["ones"][:])
        t_ob = Pp.tile([128, TK], F32, tag="obias")
        nc.sync.dma_start(t_ob[:], d["obias"][:])

        t_emb = Pp.tile([128, HK, R], BF, tag="emb")
        t_ge = Pp.tile([128, G, R], BF, tag="ge")
        t_h = [Pp.tile([128, HK, R], BF, tag=f"hT{i}", name=f"hT{i}")
               for i in (0, 1)]
        t_c = Pp.tile([128, HK, R], F32, tag="c")
        t_note = Pp.tile([128, TK, R], BF, tag="note")
        nc.sync.dma_start(t_h[0][:], d["h0T"][:])
        nc.sync.dma_start(t_c[:], d["c0T"][:])

        # ---------------- conductor ----------------
        with tc.tile_pool(name="cond", bufs=1) as Pc, \
             tc.tile_pool(name="ctmp", bufs=2) as Pt, \
             tc.tile_pool(name="cps", bufs=4, space="PSUM") as PSc, \
             tc.tile_pool(name="gzps", bufs=2, space="PSUM") as PSz:
            t_gz = Pc.tile([128, G, R], BF, tag="gz")
            t_cc = Pc.tile([128, HK, Bc], F32, tag="cc")

            # gz = z @ cond_Wih.T + cond_b for all levels at once
            with tc.tile_pool(name="condA", bufs=1) as Pca:
                t_cb = Pca.tile([1, 4 * H], BF, tag="cbias")
                nc.sync.dma_start(t_cb[:], d["cbias"][:])
                t_cwih = Pca.tile([128, ZK, 4 * H], BF, tag="cwih")
                nc.sync.dma_start(t_cwih[:], d["cwih"][:])
                t_zT = Pca.tile([128, ZK, R], BF, tag="zT")
                nc.sync.dma_start(t_zT[:], d["zT"][:])
                for m in range(G):
                    ms = slice(m * 128, (m + 1) * 128)
                    ps = PSz.tile([128, R], F32, tag="gzp")
                    nc.tensor.matmul(ps[:], t_cb[0:1, ms], t_ones[:],
                                     start=True, stop=False)
                    for k in range(ZK):
                        nc.tensor.matmul(ps[:], t_cwih[:, k, ms],
                                         t_zT[:, k, :],
                                         start=False, stop=(k == ZK - 1))
                    nc.vector.tensor_copy(t_gz[:, m, :], ps[:])

            # sequential levels
            with tc.tile_pool(name="condB", bufs=1) as Pcb:
                t_cwhh = Pcb.tile([128, HK, 4 * H], BF, tag="cwhh")
                nc.sync.dma_start(t_cwhh[:], d["cwhh"][:])
                for lv in range(L):
                    cs = slice(lv * Bc, (lv + 1) * Bc)
                    ps_prev = slice((lv - 1) * Bc, lv * Bc)
                    for p in range(HK):
                        ps = PSc.tile([128, 4, Bc], F32, tag="cgp")
                        for gi, m in enumerate((p, HK + p, 2 * HK + p,
                                                3 * HK + p)):
                            ms = slice(m * 128, (m + 1) * 128)
                            nc.tensor.matmul(ps[:, gi, :], t_ident[:],
                                             t_gz[:, m, cs],
                                             start=True, stop=(lv == 0))
                            if lv > 0:
                                for k in range(HK):
                                    nc.tensor.matmul(
                                        ps[:, gi, :], t_cwhh[:, k, ms],
                                        t_emb[:, k, ps_prev],
                                        start=False, stop=(k == HK - 1))
                        ti = Pt.tile([128, Bc], BF, tag="ti")
                        tf = Pt.tile([128, Bc], BF, tag="tf")
                        tg = Pt.tile([128, Bc], BF, tag="tg")
                        to = Pt.tile([128, Bc], BF, tag="to")
                        tcn = Pt.tile([128, Bc], BF, tag="tcn")
                        tm1 = Pt.tile([128, Bc], BF, tag="tm1")
                        tm2 = Pt.tile([128, Bc], F32, tag="tm2")
                        nc.scalar.activation(ti[:], ps[:, 0, :], AF.Sigmoid)
                        nc.scalar.activation(tf[:], ps[:, 1, :], AF.Sigmoid)
                        nc.scalar.activation(tg[:], ps[:, 2, :], AF.Tanh)
                        nc.scalar.activation(to[:], ps[:, 3, :], AF.Sigmoid)
                        nc.vector.tensor_mul(tm1[:], ti[:], tg[:])
                        if lv == 0:
                            nc.vector.tensor_copy(t_cc[:, p, :], tm1[:])
                        else:
                            nc.vector.tensor_mul(tm2[:], tf[:], t_cc[:, p, :])
                            nc.vector.tensor_add(t_cc[:, p, :], tm1[:], tm2[:])
                        nc.scalar.activation(tcn[:], t_cc[:, p, :], AF.Tanh)
                        nc.vector.tensor_mul(t_emb[:, p, cs], to[:], tcn[:])

        # decoder weights: load while ge phase computes
        Pw = ctx.enter_context(tc.tile_pool(name="wdec", bufs=1))
        t_dwn = Pw.tile([128, TK, 4 * H], BF, tag="dwn")
        nc.sync.dma_start(t_dwn[:], d["dwn"][:])
        t_dwhh = Pw.tile([128, HK, 4 * H], BF, tag="dwhh")
        nc.sync.dma_start(t_dwhh[:], d["dwhh"][:])
        t_owt = Pw.tile([128, HK, T], BF, tag="owt")
        nc.sync.dma_start(t_owt[:], d["owt"][:])

        # ---------------- ge = emb @ dec_Wih[:, :H].T + dec_b ----------------
        with tc.tile_pool(name="gew", bufs=8) as Pgw, \
             tc.tile_pool(name="geps", bufs=2, space="PSUM") as PSg:
            t_dbias = Pgw.tile([1, 4 * H], BF, tag="dbias", bufs=1)
            nc.sync.dma_start(t_dbias[:], d["dbias"][:])
            for m in range(G):
                ms = slice(m * 128, (m + 1) * 128)
                ps = PSg.tile([128, R], F32, tag="gep")
                nc.tensor.matmul(ps[:], t_dbias[0:1, ms], t_ones[:],
                                 start=True, stop=False)
                for k in range(HK):
                    wt = Pgw.tile([128, 128], BF, tag="dwe")
                    nc.sync.dma_start(wt[:], d["dwe"][k, m])
                    nc.tensor.matmul(ps[:], wt[:], t_emb[:, k, :],
                                     start=False, stop=(k == HK - 1))
                nc.vector.tensor_copy(t_ge[:, m, :], ps[:])

        # ---------------- decoder: 16 note steps over 512 rows --------------
        with tc.tile_pool(name="dtmp", bufs=2) as Pdt, \
             tc.tile_pool(name="dps", bufs=2, space="PSUM") as PSd:
            for t in range(NS):
                hin = t_h[t % 2]
                hout = t_h[(t + 1) % 2]
                for p in range(HK):
                    ps = PSd.tile([128, 4, R], F32, tag="dgp")
                    for gi, m in enumerate((p, HK + p, 2 * HK + p,
                                            3 * HK + p)):
                        ms = slice(m * 128, (m + 1) * 128)
                        nc.tensor.matmul(ps[:, gi, :], t_ident[:],
                                         t_ge[:, m, :], start=True, stop=False)
                        if t > 0:
                            for k in range(TK):
                                nc.tensor.matmul(
                                    ps[:, gi, :], t_dwn[:, k, ms],
                                    t_note[:, k, :], start=False, stop=False)
                        for k in range(HK):
                            nc.tensor.matmul(
                                ps[:, gi, :], t_dwhh[:, k, ms],
                                hin[:, k, :], start=False, stop=(k == HK - 1))
                    ti = Pdt.tile([128, R], BF, tag="ti")
                    tf = Pdt.tile([128, R], BF, tag="tf")
                    tg = Pdt.tile([128, R], BF, tag="tg")
                    to = Pdt.tile([128, R], BF, tag="to")
                    tcn = Pdt.tile([128, R], BF, tag="tcn")
                    tm1 = Pdt.tile([128, R], BF, tag="tm1")
                    tm2 = Pdt.tile([128, R], F32, tag="tm2")
                    nc.scalar.activation(ti[:], ps[:, 0, :], AF.Sigmoid)
                    nc.scalar.activation(tf[:], ps[:, 1, :], AF.Sigmoid)
                    nc.scalar.activation(tg[:], ps[:, 2, :], AF.Tanh)
                    nc.scalar.activation(to[:], ps[:, 3, :], AF.Sigmoid)
                    nc.vector.tensor_mul(tm1[:], ti[:], tg[:])
                    nc.vector.tensor_mul(tm2[:], tf[:], t_c[:, p, :])
                    nc.vector.tensor_add(t_c[:, p, :], tm1[:], tm2[:])
                    nc.scalar.activation(tcn[:], t_c[:, p, :], AF.Tanh)
                    nc.vector.tensor_mul(hout[:, p, :], to[:], tcn[:])
                # output projection + sigmoid -> note (bf16, also the output)
                po = PSd.tile([128, 4, R], F32, tag="dgp")
                for tk in range(TK):
                    ts_ = slice(tk * 128, (tk + 1) * 128)
                    for k in range(HK):
                        nc.tensor.matmul(po[:, tk, :], t_owt[:, k, ts_],
                                         hout[:, k, :],
                                         start=(k == 0), stop=(k == HK - 1))
                    nc.scalar.activation(t_note[:, tk, :], po[:, tk, :],
                                         AF.Sigmoid, bias=t_ob[:, tk:tk + 1])
                    nc.sync.dma_start(d["outbuf"][t, tk], t_note[:, tk, :])


_CACHE = {}


def _build():
    if "nc" not in _CACHE:
        nc = bacc.Bacc("TRN2", target_bir_lowering=False, debug=False,
                       num_devices=NCORES)
        d = _declare(nc)
        with tile.TileContext(nc) as tc:
            _body(nc, tc, d)
        nc.compile()
        _CACHE["nc"] = nc
    return _CACHE["nc"]


def _feat_major(W):
    """[J, K] -> [128, K/128, J] bf16 (stationary lhsT chunk layout)."""
    J, K = W.shape
    return np.ascontiguousarray(
        W.reshape(J, K // 128, 128).transpose(2, 1, 0)).astype(bf16)


def _pack_inputs(inputs):
    z = np.asarray(inputs["z"], np.float32)
    dec_h0 = np.asarray(inputs["dec_h0"], np.float32)
    dec_c0 = np.asarray(inputs["dec_c0"], np.float32)
    cond_b = np.asarray(inputs["cond_bih"] + inputs["cond_bhh"], np.float32)
    dec_b = np.asarray(inputs["dec_bih"] + inputs["dec_bhh"], np.float32)
    out_b = np.asarray(inputs["out_b"], np.float32)

    shared = {
        "ident": np.eye(128, dtype=np.float32).astype(bf16),
        "ones": np.ones((1, R), dtype=bf16),
        "cbias": cond_b[None, :].astype(bf16),
        "dbias": dec_b[None, :].astype(bf16),
        "obias": np.ascontiguousarray(out_b.reshape(TK, 128).T).astype(np.float32),
        "cwih": _feat_major(np.asarray(inputs["cond_Wih"], np.float32)),
        "cwhh": _feat_major(np.asarray(inputs["cond_Whh"], np.float32)),
        "dwn": _feat_major(np.asarray(inputs["dec_Wih"][:, H:], np.float32)),
        "dwhh": _feat_major(np.asarray(inputs["dec_Whh"], np.float32)),
        "owt": _feat_major(np.asarray(inputs["out_W"], np.float32)),
    }
    dwe_fm = _feat_major(np.asarray(inputs["dec_Wih"][:, :H], np.float32))
    shared["dwe"] = np.ascontiguousarray(
        dwe_fm.reshape(128, HK, G, 128).transpose(1, 2, 0, 3))

    z_lv = z[:, np.arange(L) * L, 0, :]           # [B, L, Z]
    in_maps = []
    for c in range(NCORES):
        bs = slice(c * Bc, (c + 1) * Bc)
        zc = z_lv[bs]                              # [Bc, L, Z]
        zT = np.ascontiguousarray(
            zc.reshape(Bc, L, ZK, 128).transpose(3, 2, 1, 0).reshape(128, ZK, R)
        ).astype(bf16)
        h0 = dec_h0[:, bs, :]                      # [L, Bc, H]
        h0T = np.ascontiguousarray(
            h0.reshape(L, Bc, HK, 128).transpose(3, 2, 0, 1).reshape(128, HK, R))
        c0 = dec_c0[:, bs, :]
        c0T = np.ascontiguousarray(
            c0.reshape(L, Bc, HK, 128).transpose(3, 2, 0, 1).reshape(128, HK, R))
        m = dict(shared)
        m["zT"] = zT
        m["h0T"] = h0T.astype(bf16)
        m["c0T"] = c0T.astype(np.float32)
        in_maps.append(m)
    return in_maps


def _unpack_outputs(core_outs):
    notes = np.empty((B, L * NS, T), np.float32)
    for c, arr in enumerate(core_outs):
        # arr [NS, TK, 128, R] -> [Bc, L, NS, T]
        a = arr.astype(np.float32).reshape(NS, TK, 128, L, Bc).transpose(4, 3, 0, 1, 2)
        notes[c * Bc:(c + 1) * Bc] = a.reshape(Bc, L, NS, T).reshape(
            Bc, L * NS, T)
    return notes


def kernel(**inputs):
    nc = _build()
    in_maps = _pack_inputs(inputs)
    res = run_bass_kernel_spmd(nc, in_maps, list(range(NCORES)))
    return _unpack_outputs([r["outbuf"] for r in res.results])
